# revision 1
# baseline (speedup 1.0000x reference)
"""Trainium2 Bass kernel for the attention-LSTM decoder (B=256, T-1=32, ENC=DEC=128, OUT=1).

Sharding: data-parallel, batch 256 -> 32 per core across 8 cores. The T-1=32
recurrence runs locally per core, fully unrolled.

Per-core layouts (Bs=32, tau-major free index j = tau*32 + b):
  - states: H = 2*h [128(dec), 32(b)] bf16, C [128, 32] f32 (+bf16 copy for matmul rhs)
  - P = W1_enc @ enc + b1 precomputed once: [128(h), 1024(j)] bf16
  - per step: q = W1_h@h + W1_c@c (PE) -> pre = P + bcast(q) (DVE) -> hdn = tanh (ACT)
    -> logits via 8 MMs (lhsT = hdn chunk, rhs = W2) into PSUM [128(r), 8(k)]
    -> E = exp (ACT, fused row-sum) -> S = SEL4^T-matmul partition-sum -> 1/S (DVE)
    -> replicate back via SEL4-matmul -> masked attn matrix (fused DVE stt)
    -> ctx via 8 accumulating MMs (lhsT = enc natural chunks)
    -> gates = [0.5*Whh | Wc | wy;bias] @ [H; ctx; y_t;1] (12 MMs, gate order g,i,f,o)
    -> tanh-only LSTM pointwise (sigmoid(x) == 0.5*(1+tanh(x/2)), no table switch)
"""

import os

import numpy as np
import ml_dtypes

_PROBE = os.environ.get("KPROBE", "")  # "noB" / "noC" cost-attribution probes

import concourse.bass as bass
import concourse.bacc as bacc
import concourse.tile as tile
from concourse import mybir
from concourse.bass_utils import run_bass_kernel_spmd

F32 = mybir.dt.float32
BF16 = mybir.dt.bfloat16
AF = mybir.ActivationFunctionType
OP = mybir.AluOpType

B, T, ENC, DEC = 256, 32, 128, 128
NCORES = 8
BS = B // NCORES  # 32 batch rows per core


def _ap_with(ap_obj, dims):
    """Build an AP with explicit free dims (list of [step, count]) keeping partition dim."""
    return bass.AP(tensor=ap_obj.tensor, offset=ap_obj.offset, ap=[ap_obj.ap[0]] + dims)


def build_program(n_steps=T):
    # Bacc (not plain Bass): its compile() runs move_matmul_waits_to_ldweights +
    # generate_event_semaphores, required because HW instructions hold only ONE
    # semaphore wait each.
    nc = bacc.Bacc()

    # ---- DRAM I/O (per-core shard, host-prepared layouts) ----
    d_encT = nc.dram_tensor("encT", [ENC, T * BS], F32, kind="ExternalInput")
    d_encN = nc.dram_tensor("encN", [128, 8 * ENC], BF16, kind="ExternalInput")
    d_yaug = nc.dram_tensor("yaug", [2, T * BS], BF16, kind="ExternalInput")
    d_ylast = nc.dram_tensor("ylast", [1, BS], F32, kind="ExternalInput")
    d_w1eT = nc.dram_tensor("w1eT", [ENC, 128], F32, kind="ExternalInput")
    d_b1 = nc.dram_tensor("b1", [128, 1], F32, kind="ExternalInput")
    d_w1hT = nc.dram_tensor("w1hT", [DEC, 128], BF16, kind="ExternalInput")
    d_w1cT = nc.dram_tensor("w1cT", [DEC, 128], BF16, kind="ExternalInput")
    d_w2c = nc.dram_tensor("w2c", [128, 1], BF16, kind="ExternalInput")
    d_sel4rep = nc.dram_tensor("sel4rep", [128, 128], F32, kind="ExternalInput")
    d_sel4b = nc.dram_tensor("sel4b", [128, BS], BF16, kind="ExternalInput")
    d_whhT = nc.dram_tensor("whhT", [DEC, 512], BF16, kind="ExternalInput")
    d_wcT = nc.dram_tensor("wcT", [ENC, 512], BF16, kind="ExternalInput")
    d_waug = nc.dram_tensor("waug", [2, 512], BF16, kind="ExternalInput")
    d_fcf = nc.dram_tensor("fcf", [128, 2], BF16, kind="ExternalInput")
    d_fcfb = nc.dram_tensor("fcfb", [1, 1], F32, kind="ExternalInput")
    d_out = nc.dram_tensor("outp", [1, BS], F32, kind="ExternalOutput")

    with tile.TileContext(nc) as tc:
        with (
            tc.tile_pool(name="consts", bufs=1) as consts,
            tc.tile_pool(name="state", bufs=1) as state,
            tc.tile_pool(name="temps", bufs=3) as temps,
            tc.tile_pool(name="psum", bufs=1, space="PSUM") as psum,
        ):
            # ---- load constants ----
            encN = consts.tile([128, 8 * ENC], BF16)
            nc.sync.dma_start(out=encN, in_=d_encN[:, :])
            yaug = consts.tile([2, T * BS], BF16)
            nc.sync.dma_start(out=yaug, in_=d_yaug[:, :])
            ylast = consts.tile([1, BS], F32)
            nc.sync.dma_start(out=ylast, in_=d_ylast[:, :])
            b1 = consts.tile([128, 1], F32)
            nc.sync.dma_start(out=b1, in_=d_b1[:, :])
            w1hT = consts.tile([DEC, 128], BF16)
            nc.sync.dma_start(out=w1hT, in_=d_w1hT[:, :])
            w1cT = consts.tile([DEC, 128], BF16)
            nc.sync.dma_start(out=w1cT, in_=d_w1cT[:, :])
            w2c = consts.tile([128, 1], BF16)
            nc.sync.dma_start(out=w2c, in_=d_w2c[:, :])
            sel4rep = consts.tile([128, 128], F32)
            nc.sync.dma_start(out=sel4rep, in_=d_sel4rep[:, :])
            sel4b = consts.tile([128, BS], BF16)
            nc.sync.dma_start(out=sel4b, in_=d_sel4b[:, :])
            whhT = consts.tile([DEC, 512], BF16)
            nc.sync.dma_start(out=whhT, in_=d_whhT[:, :])
            wcT = consts.tile([ENC, 512], BF16)
            nc.sync.dma_start(out=wcT, in_=d_wcT[:, :])
            waug = consts.tile([2, 512], BF16)
            nc.sync.dma_start(out=waug, in_=d_waug[:, :])
            fcf = consts.tile([128, 2], BF16)
            nc.sync.dma_start(out=fcf, in_=d_fcf[:, :])

            # ---- prologue: P = W1_enc @ enc + b1  -> bf16 [128, 1024] ----
            P = consts.tile([128, T * BS], BF16)
            with tc.tile_pool(name="prolog", bufs=1) as prolog:
                encT = prolog.tile([ENC, T * BS], F32)
                nc.sync.dma_start(out=encT, in_=d_encT[:, :])
                w1eT = prolog.tile([ENC, 128], F32)
                nc.sync.dma_start(out=w1eT, in_=d_w1eT[:, :])

                # PE sync-fence: walrus Matmult/LDWEIGHTS carries at most ONE
                # semaphore wait. Touch every DMA-loaded tile with a dummy
                # 1x1x1 matmul (both operands in the same tile -> 1 wait each)
                # so no real matmul is first-contact for two sem domains.
                pdum = psum.tile([1, 1], F32, tag="S")
                dscr = prolog.tile([1, 16], F32)
                for i, cst in enumerate((encT, w1eT, encN, yaug, ylast, b1, w1hT,
                                         w1cT, w2c, sel4rep, sel4b, whhT,
                                         wcT, waug, fcf)):
                    nc.tensor.matmul(pdum[:, :], cst[0:1, 0:1], cst[0:1, 0:1],
                                     start=True, stop=True)
                    # same fence for the vector engine (1-wait limit is universal)
                    nc.vector.tensor_copy(dscr[0:1, i:i + 1], cst[0:1, 0:1])

                for half in range(2):
                    pP = psum.tile([128, 512], F32, tag="gbank")
                    nc.tensor.matmul(
                        pP[:, :], w1eT[:, :], encT[:, half * 512:(half + 1) * 512],
                        start=True, stop=True,
                    )
                    # P half = psum + b1 (per-partition scalar), cast to bf16
                    nc.vector.tensor_scalar(
                        out=P[:, half * 512:(half + 1) * 512],
                        in0=pP[:, :], scalar1=b1[:, :], scalar2=None, op0=OP.add,
                    )

            # ---- state init ----
            H = state.tile([DEC, BS], BF16)   # 2*h
            Cn = state.tile([DEC, BS], F32)   # c
            Cb = state.tile([DEC, BS], BF16)  # bf16 copy of c
            nc.vector.memset(H, 0.0)
            nc.vector.memset(Cn, 0.0)
            nc.vector.memset(Cb, 0.0)

            ctx_sb = None
            for t in range(n_steps):
                # --- phase A: attention MLP ---
                pq = psum.tile([128, BS], F32, tag="q")
                nc.tensor.matmul(pq[:, :], w1cT[:, :], Cb[:, :], start=True, stop=False)
                nc.tensor.matmul(pq[:, :], w1hT[:, :], H[:, :], start=False, stop=True)
                q_sb = temps.tile([128, BS], BF16, tag="qsb")
                nc.vector.tensor_copy(q_sb[:, :], pq[:, :])

                # halves pipeline: DVE pre-add h1 overlaps ACT tanh h0;
                # logits MMs for chunks 0-3 overlap ACT tanh h1
                pre = temps.tile([128, T * BS], BF16, tag="pre")
                hdn = temps.tile([128, T * BS], BF16, tag="hdn")
                pL = psum.tile([128, 8], F32, tag="L")
                HW_ = T * BS // 2  # 512 elems = 4 chunks per half
                for h in range(2):
                    q_b = _ap_with(q_sb[:, :], [[0, T // 2], [1, BS]])
                    nc.vector.tensor_add(
                        pre[:, h * HW_:(h + 1) * HW_].rearrange("p (t b) -> p t b", b=BS),
                        P[:, h * HW_:(h + 1) * HW_].rearrange("p (t b) -> p t b", b=BS),
                        q_b,
                    )
                    nc.scalar.activation(hdn[:, h * HW_:(h + 1) * HW_],
                                         pre[:, h * HW_:(h + 1) * HW_], AF.Tanh)
                    for k in range(4 * h, 4 * h + 4):
                        nc.tensor.matmul(
                            pL[:, k:k + 1], hdn[:, k * 128:(k + 1) * 128], w2c[:, :],
                            start=True, stop=True,
                        )
                # --- phase B: softmax + context ---
                if _PROBE == "noB":
                    ctx_sb = temps.tile([128, BS], BF16, tag="ctxsb")
                    nc.scalar.copy(out=ctx_sb[:, :], in_=pq[:, :])
                E2 = temps.tile([128, 8], BF16, tag="E2")
                Ered = temps.tile([128, 1], F32, tag="Ered")
                if _PROBE != "noB":
                    nc.scalar.activation(E2[:, :], pL[:, :], AF.Exp, accum_out=Ered[:, :])
                if _PROBE != "noB":
                    # replicate+sum in ONE matmul: S128[p] = sum_r [r%32==p%32]*Ered[r]
                    pS = psum.tile([128, 1], F32, tag="S")
                    nc.tensor.matmul(pS[:, :], sel4rep[:, :], Ered[:, :], start=True, stop=True)
                    R128 = temps.tile([128, 1], F32, tag="R128")
                    nc.vector.reciprocal(R128[:, :], pS[:, :])

                    # unnormalized masked attn matrix: independent of S/recip,
                    # overlaps the S-matmul + reciprocal on the other engines
                    abuf_u = temps.tile([128, 8 * BS], BF16, tag="abufu")
                    e2_b = _ap_with(E2[:, :], [[1, 8], [0, BS]])
                    sel_b = _ap_with(sel4b[:, :], [[0, 8], [1, BS]])
                    nc.vector.tensor_mul(
                        abuf_u[:, :].rearrange("p (k b) -> p k b", b=BS),
                        e2_b, sel_b,
                    )
                    # normalize rows by 1/S: cheap per-partition tensor_scalar
                    abuf = temps.tile([128, 8 * BS], BF16, tag="abuf")
                    nc.vector.tensor_scalar(
                        out=abuf[:, :], in0=abuf_u[:, :], scalar1=R128[:, :],
                        scalar2=None, op0=OP.mult,
                    )
                    pctx = psum.tile([128, BS], F32, tag="ctx")
                    for k in range(8):
                        nc.tensor.matmul(
                            pctx[:, :], encN[:, k * 128:(k + 1) * 128],
                            abuf[:, k * BS:(k + 1) * BS],
                            start=(k == 0), stop=(k == 7),
                        )
                    ctx_sb = temps.tile([128, BS], BF16, tag="ctxsb")
                    nc.scalar.copy(out=ctx_sb[:, :], in_=pctx[:, :])

                # --- phase C: gates + LSTM pointwise ---
                pg = psum.tile([128, 4 * BS], F32, tag="g")
                for m in range(4):
                    sl = pg[:, m * BS:(m + 1) * BS]
                    nc.tensor.matmul(sl, whhT[:, m * 128:(m + 1) * 128], H[:, :],
                                     start=True, stop=False)
                    nc.tensor.matmul(sl, wcT[:, m * 128:(m + 1) * 128], ctx_sb[:, :],
                                     start=False, stop=False)
                    nc.tensor.matmul(sl, waug[:, m * 128:(m + 1) * 128],
                                     yaug[:, t * BS:(t + 1) * BS],
                                     start=False, stop=True)
                tifo = temps.tile([128, 3 * BS], F32, tag="tifo")
                nc.scalar.activation(tifo[:, 0:2 * BS], pg[:, BS:3 * BS],
                                     AF.Tanh, scale=0.5)  # t_i, t_f
                gt = temps.tile([128, BS], F32, tag="gt")
                nc.scalar.activation(gt[:, :], pg[:, 0:BS], AF.Tanh)
                nc.scalar.activation(tifo[:, 2 * BS:3 * BS], pg[:, 3 * BS:4 * BS],
                                     AF.Tanh, scale=0.5)  # t_o
                v = temps.tile([128, BS], F32, tag="v")
                nc.vector.scalar_tensor_tensor(
                    out=v[:, :], in0=tifo[:, BS:2 * BS], scalar=1.0, in1=Cn[:, :],
                    op0=OP.add, op1=OP.mult)  # (t_f+1)*c = 2*sig(f)*c
                u = temps.tile([128, BS], F32, tag="u")
                nc.vector.scalar_tensor_tensor(
                    out=u[:, :], in0=tifo[:, 0:BS], scalar=1.0, in1=gt[:, :],
                    op0=OP.add, op1=OP.mult)  # (t_i+1)*g~ = 2*sig(i)*g~
                w2 = temps.tile([128, BS], F32, tag="w2t")
                nc.vector.tensor_add(w2[:, :], u[:, :], v[:, :])  # 2*c_new
                # tanh(c') straight from 2c' (scale=0.5); Cn/Cb updates run
                # off the critical chain in parallel with th/H
                th = temps.tile([128, BS], F32, tag="th")
                nc.scalar.activation(th[:, :], w2[:, :], AF.Tanh, scale=0.5)
                nc.vector.tensor_scalar(out=Cn[:, :], in0=w2[:, :], scalar1=0.5,
                                        scalar2=None, op0=OP.mult)
                nc.vector.tensor_scalar(out=Cb[:, :], in0=w2[:, :], scalar1=0.5,
                                        scalar2=None, op0=OP.mult)
                nc.vector.scalar_tensor_tensor(
                    out=H[:, :], in0=tifo[:, 2 * BS:3 * BS], scalar=1.0, in1=th[:, :],
                    op0=OP.add, op1=OP.mult)  # (t_o+1)*tanh(c) = 2*h_new

            # ---- final output ----
            po = psum.tile([1, BS], F32, tag="o")
            nc.tensor.matmul(po[:, :], fcf[:, 0:1], H[:, :], start=True, stop=False)
            nc.tensor.matmul(po[:, :], fcf[:, 1:2], ctx_sb[:, :], start=False, stop=True)
            fcfb = consts.tile([1, 1], F32)
            nc.sync.dma_start(out=fcfb, in_=d_fcfb[:, :])
            out_sb = temps.tile([1, BS], F32, tag="osb")
            nc.vector.scalar_tensor_tensor(
                out=out_sb[:, :], in0=po[:, :], scalar=fcfb[:, :], in1=ylast[:, :],
                op0=OP.add, op1=OP.add)
            nc.sync.dma_start(out=d_out[:, :], in_=out_sb[:, :])

    nc.compile()
    return nc


def _prep_inputs(input_encoded, y_history, attn_W1, attn_b1, attn_W2, attn_b2,
                 W_ih, W_hh, b_ih, b_hh, fc_W, fc_b, fcf_W, fcf_b):
    """Host-side weight fusion + per-core shard layout prep (numpy only)."""
    f32 = np.float32
    bf16 = ml_dtypes.bfloat16
    input_encoded = np.asarray(input_encoded, f32)
    y_history = np.asarray(y_history, f32)

    # attention weights
    W1 = np.asarray(attn_W1, f32)            # [128, 384] cols: h, c, enc
    w1hT = np.ascontiguousarray((0.5 * W1[:, 0:128]).T)     # H = 2h
    w1cT = np.ascontiguousarray(W1[:, 128:256].T)
    w1eT = np.ascontiguousarray(W1[:, 256:384].T)
    b1 = np.asarray(attn_b1, f32).reshape(128, 1)
    w2c = np.asarray(attn_W2, f32).reshape(1, 128).T.copy()  # [128,1]

    # fused gate weights; reorder (i,f,g,o) -> (g,i,f,o)
    W_ih = np.asarray(W_ih, f32)
    W_hh = np.asarray(W_hh, f32)
    fc_W = np.asarray(fc_W, f32)
    wc_full = np.outer(W_ih[:, 0], fc_W[0, :128])            # [512, 128]
    w_y = W_ih[:, 0] * fc_W[0, 128]
    bias_g = np.asarray(b_ih, f32) + np.asarray(b_hh, f32) + W_ih[:, 0] * f32(fc_b[0])
    perm = np.r_[256:384, 0:128, 128:256, 384:512]
    whhT = np.ascontiguousarray((0.5 * W_hh[perm]).T)        # [128, 512]
    wcT = np.ascontiguousarray(wc_full[perm].T)              # [128, 512]
    waug = np.stack([w_y[perm], bias_g[perm]], 0)            # [2, 512]

    fcf_W = np.asarray(fcf_W, f32)
    fcf = np.stack([0.5 * fcf_W[0, 0:128], fcf_W[0, 128:256]], 1)  # [128, 2]
    fcfb = np.array([[np.asarray(fcf_b, f32).reshape(-1)[0]]], f32)

    # selection matrices: sel4[r, b] = (r % 32 == b); sel4rep[r, p] = (r%32 == p%32)
    r = np.arange(128)
    sel4 = (np.equal.outer(r % BS, np.arange(BS))).astype(f32)  # [128, 32]
    sel4rep = (np.equal.outer(r % BS, np.arange(128) % BS)).astype(f32)  # [128, 128]

    shared = dict(
        w1eT=w1eT, b1=b1,
        w1hT=w1hT.astype(bf16), w1cT=w1cT.astype(bf16), w2c=w2c.astype(bf16),
        sel4rep=sel4rep, sel4b=sel4.astype(bf16),
        whhT=whhT.astype(bf16), wcT=wcT.astype(bf16), waug=waug.astype(bf16),
        fcf=fcf.astype(bf16), fcfb=fcfb,
    )

    in_maps = []
    for c in range(NCORES):
        enc_c = input_encoded[c * BS:(c + 1) * BS]           # [32, 32, 128]
        y_c = y_history[c * BS:(c + 1) * BS, :, 0]           # [32b, 32tau]
        encT = np.ascontiguousarray(enc_c.transpose(2, 1, 0).reshape(ENC, T * BS))
        # encN[r, k*128+e] = enc[b=r%32, tau=4k+r//32, e]
        tmp = enc_c.transpose(1, 0, 2).reshape(8, 4, BS, ENC)   # [k, tau_lo, b, e]
        encN = np.ascontiguousarray(tmp.transpose(1, 2, 0, 3).reshape(128, 8 * ENC))
        yrow = np.ascontiguousarray(y_c.T.reshape(1, T * BS))   # [1, tau*32+b]
        yaug = np.concatenate([yrow, np.ones_like(yrow)], 0)    # [2, 1024]
        m = dict(shared)
        m.update(
            encT=encT, encN=encN.astype(bf16), yaug=yaug.astype(bf16),
            ylast=np.ascontiguousarray(y_c[:, T - 1].reshape(1, BS)),
        )
        in_maps.append(m)
    return in_maps


_CACHED = {}


def kernel(**inputs) -> np.ndarray:
    in_maps = _prep_inputs(**inputs)
    if "nc" not in _CACHED:
        _CACHED["nc"] = build_program()
    res = run_bass_kernel_spmd(_CACHED["nc"], in_maps, core_ids=list(range(NCORES)))
    out = np.concatenate([r["outp"].reshape(BS, 1) for r in res.results], 0)
    return out.astype(np.float32)


if __name__ == "__main__":
    import reference
    inputs = {k: np.asarray(v) for k, v in reference.setup_inputs().items()}
    expected = np.asarray(reference.reference(**inputs))
    actual = kernel(**inputs)
    err = np.abs(actual - expected).max() / (np.abs(expected).max() + 1e-12)
    print("Relative error:", err)



# revision 2
# speedup vs baseline: 1.1041x; 1.1041x over previous
"""Trainium2 Bass kernel for the attention-LSTM decoder (B=256, T-1=32, ENC=DEC=128, OUT=1).

Sharding: data-parallel, batch 256 -> 32 per core across 8 cores. The T-1=32
recurrence runs locally per core, fully unrolled.

Per-core layouts (Bs=32, tau-major free index j = tau*32 + b):
  - states: H = 2*h [128(dec), 32(b)] bf16, C [128, 32] f32 (+bf16 copy for matmul rhs)
  - P = W1_enc @ enc + b1 precomputed once: [128(h), 1024(j)] bf16
  - per step: q = W1_h@h + W1_c@c (PE) -> pre = P + bcast(q) (DVE) -> hdn = tanh (ACT)
    -> logits via 8 MMs (lhsT = hdn chunk, rhs = W2) into PSUM [128(r), 8(k)]
    -> E = exp (ACT) -> S replicated to [128, 32] via 8 bcast-stationary MMs
    -> unnormalized ctx via 8 accumulating MMs over abuf = E (x) sel mask
    -> ctx = pctx * (1/S) (DVE, normalize-late)
    -> gates = [Whh' | Wc' | waug'] @ [H; ctx; y_t;1] (12 MMs, order g,i,f,o;
       the 0.5 sigmoid-as-tanh scale for i,f,o is folded into the weights)
    -> tanh-only LSTM pointwise (sigmoid(x) == 0.5*(1+tanh(x/2)))
"""

import numpy as np
import ml_dtypes

import concourse.bass as bass
import concourse.bacc as bacc
import concourse.tile as tile
from concourse import mybir
from concourse.bass_utils import run_bass_kernel_spmd

F32 = mybir.dt.float32
BF16 = mybir.dt.bfloat16
AF = mybir.ActivationFunctionType
OP = mybir.AluOpType

B, T, ENC, DEC = 256, 32, 128, 128
NCORES = 8
BS = B // NCORES  # 32 batch rows per core


def _ap_with(ap_obj, dims):
    """Build an AP with explicit free dims (list of [step, count]) keeping partition dim."""
    return bass.AP(tensor=ap_obj.tensor, offset=ap_obj.offset, ap=[ap_obj.ap[0]] + dims)


def build_program(n_steps=T):
    # Bacc (not plain Bass): its compile() runs move_matmul_waits_to_ldweights +
    # generate_event_semaphores, required because HW instructions hold only ONE
    # semaphore wait each.
    nc = bacc.Bacc()

    # ---- DRAM I/O (per-core shard, host-prepared layouts) ----
    d_encT = nc.dram_tensor("encT", [ENC, T * BS], F32, kind="ExternalInput")
    d_encN = nc.dram_tensor("encN", [128, 8 * ENC], BF16, kind="ExternalInput")
    d_yaug = nc.dram_tensor("yaug", [2, T * BS], BF16, kind="ExternalInput")
    d_ylast = nc.dram_tensor("ylast", [1, BS], F32, kind="ExternalInput")
    d_w1eT = nc.dram_tensor("w1eT", [ENC, 128], F32, kind="ExternalInput")
    d_b1 = nc.dram_tensor("b1", [128, 1], F32, kind="ExternalInput")
    d_w1hT = nc.dram_tensor("w1hT", [DEC, 128], BF16, kind="ExternalInput")
    d_w1cT = nc.dram_tensor("w1cT", [DEC, 128], BF16, kind="ExternalInput")
    d_w2c = nc.dram_tensor("w2c", [128, 1], BF16, kind="ExternalInput")
    d_sel4b = nc.dram_tensor("sel4b", [128, BS], BF16, kind="ExternalInput")
    d_whhT = nc.dram_tensor("whhT", [DEC, 512], BF16, kind="ExternalInput")
    d_wcT = nc.dram_tensor("wcT", [ENC, 512], BF16, kind="ExternalInput")
    d_waug = nc.dram_tensor("waug", [2, 512], BF16, kind="ExternalInput")
    d_fcf = nc.dram_tensor("fcf", [128, 2], BF16, kind="ExternalInput")
    d_fcfb = nc.dram_tensor("fcfb", [1, 1], F32, kind="ExternalInput")
    d_out = nc.dram_tensor("outp", [1, BS], F32, kind="ExternalOutput")

    with tile.TileContext(nc) as tc:
        with (
            tc.tile_pool(name="consts", bufs=1) as consts,
            tc.tile_pool(name="state", bufs=1) as state,
            tc.tile_pool(name="temps", bufs=3) as temps,
            tc.tile_pool(name="psum", bufs=1, space="PSUM") as psum,
        ):
            # ---- load constants ----
            encN = consts.tile([128, 8 * ENC], BF16)
            nc.sync.dma_start(out=encN, in_=d_encN[:, :])
            yaug = consts.tile([2, T * BS], BF16)
            nc.sync.dma_start(out=yaug, in_=d_yaug[:, :])
            ylast = consts.tile([1, BS], F32)
            nc.sync.dma_start(out=ylast, in_=d_ylast[:, :])
            b1 = consts.tile([128, 1], F32)
            nc.sync.dma_start(out=b1, in_=d_b1[:, :])
            w1hT = consts.tile([DEC, 128], BF16)
            nc.sync.dma_start(out=w1hT, in_=d_w1hT[:, :])
            w1cT = consts.tile([DEC, 128], BF16)
            nc.sync.dma_start(out=w1cT, in_=d_w1cT[:, :])
            w2c = consts.tile([128, 1], BF16)
            nc.sync.dma_start(out=w2c, in_=d_w2c[:, :])
            sel4b = consts.tile([128, BS], BF16)
            nc.sync.dma_start(out=sel4b, in_=d_sel4b[:, :])
            whhT = consts.tile([DEC, 512], BF16)
            nc.sync.dma_start(out=whhT, in_=d_whhT[:, :])
            wcT = consts.tile([ENC, 512], BF16)
            nc.sync.dma_start(out=wcT, in_=d_wcT[:, :])
            waug = consts.tile([2, 512], BF16)
            nc.sync.dma_start(out=waug, in_=d_waug[:, :])
            fcf = consts.tile([128, 2], BF16)
            nc.sync.dma_start(out=fcf, in_=d_fcf[:, :])

            # ---- prologue: P = W1_enc @ enc + b1  -> bf16 [128, 1024] ----
            P = consts.tile([128, T * BS], BF16)
            with tc.tile_pool(name="prolog", bufs=1) as prolog:
                encT = prolog.tile([ENC, T * BS], F32)
                nc.sync.dma_start(out=encT, in_=d_encT[:, :])
                w1eT = prolog.tile([ENC, 128], F32)
                nc.sync.dma_start(out=w1eT, in_=d_w1eT[:, :])

                # PE sync-fence: walrus Matmult/LDWEIGHTS carries at most ONE
                # semaphore wait. Touch every DMA-loaded tile with a dummy
                # 1x1x1 matmul (both operands in the same tile -> 1 wait each)
                # so no real matmul is first-contact for two sem domains.
                pdum = psum.tile([1, 1], F32, tag="S")
                dscr = prolog.tile([1, 16], F32)
                for i, cst in enumerate((encT, w1eT, encN, yaug, ylast, b1, w1hT,
                                         w1cT, w2c, sel4b, whhT,
                                         wcT, waug, fcf)):
                    nc.tensor.matmul(pdum[:, :], cst[0:1, 0:1], cst[0:1, 0:1],
                                     start=True, stop=True)
                    # same fence for the vector engine (1-wait limit is universal)
                    nc.vector.tensor_copy(dscr[0:1, i:i + 1], cst[0:1, 0:1])

                for half in range(2):
                    pP = psum.tile([128, 512], F32, tag="gbank")
                    nc.tensor.matmul(
                        pP[:, :], w1eT[:, :], encT[:, half * 512:(half + 1) * 512],
                        start=True, stop=True,
                    )
                    # P half = psum + b1 (per-partition scalar), cast to bf16
                    nc.vector.tensor_scalar(
                        out=P[:, half * 512:(half + 1) * 512],
                        in0=pP[:, :], scalar1=b1[:, :], scalar2=None, op0=OP.add,
                    )

            # ---- state init ----
            H = state.tile([DEC, BS], BF16)   # 2*h
            Cn = state.tile([DEC, BS], F32)   # c
            Cb = state.tile([DEC, BS], BF16)  # bf16 copy of c
            nc.vector.memset(H, 0.0)
            nc.vector.memset(Cn, 0.0)
            nc.vector.memset(Cb, 0.0)

            ctx_sb = None
            for t in range(n_steps):
                first = t == 0  # h = c = 0: skip everything scaled by h/c
                # --- phase A: attention MLP ---
                HW_ = T * BS // 2  # 512 elems = 4 chunks per half
                hdn = temps.tile([128, T * BS], BF16, tag="hdn")
                pL = psum.tile([128, 8], F32, tag="L")
                if first:
                    # q = 0 -> hdn = tanh(P)
                    for h in range(2):
                        nc.scalar.activation(hdn[:, h * HW_:(h + 1) * HW_],
                                             P[:, h * HW_:(h + 1) * HW_], AF.Tanh)
                        for k in range(4 * h, 4 * h + 4):
                            nc.tensor.matmul(
                                pL[:, k:k + 1], hdn[:, k * 128:(k + 1) * 128],
                                w2c[:, :], start=True, stop=True,
                            )
                else:
                    pq = psum.tile([128, BS], F32, tag="q")
                    nc.tensor.matmul(pq[:, :], w1cT[:, :], Cb[:, :], start=True, stop=False)
                    nc.tensor.matmul(pq[:, :], w1hT[:, :], H[:, :], start=False, stop=True)
                    q_sb = temps.tile([128, BS], BF16, tag="qsb")
                    nc.vector.tensor_copy(q_sb[:, :], pq[:, :])

                    # halves pipeline: DVE pre-add h1 overlaps ACT tanh h0;
                    # logits MMs for chunks 0-3 overlap ACT tanh h1
                    pre = temps.tile([128, T * BS], BF16, tag="pre")
                    for h in range(2):
                        q_b = _ap_with(q_sb[:, :], [[0, T // 2], [1, BS]])
                        nc.vector.tensor_add(
                            pre[:, h * HW_:(h + 1) * HW_].rearrange("p (t b) -> p t b", b=BS),
                            P[:, h * HW_:(h + 1) * HW_].rearrange("p (t b) -> p t b", b=BS),
                            q_b,
                        )
                        nc.scalar.activation(hdn[:, h * HW_:(h + 1) * HW_],
                                             pre[:, h * HW_:(h + 1) * HW_], AF.Tanh)
                        for k in range(4 * h, 4 * h + 4):
                            nc.tensor.matmul(
                                pL[:, k:k + 1], hdn[:, k * 128:(k + 1) * 128],
                                w2c[:, :], start=True, stop=True,
                            )
                # --- phase B: softmax + context, normalize-late ---
                E2 = temps.tile([128, 8], BF16, tag="E2")
                nc.scalar.activation(E2[:, :], pL[:, :], AF.Exp)
                # unnormalized masked attn matrix; overlaps the S matmuls on PE
                abuf = temps.tile([128, 8 * BS], BF16, tag="abuf")
                e2_b = _ap_with(E2[:, :], [[1, 8], [0, BS]])
                sel_b = _ap_with(sel4b[:, :], [[0, 8], [1, BS]])
                nc.vector.tensor_mul(
                    abuf[:, :].rearrange("p (k b) -> p k b", b=BS),
                    e2_b, sel_b,
                )
                # S replicated to every partition: pS[p, b] = sum_r E2[r,:] sel[r,b]
                pS = psum.tile([128, BS], F32, tag="S")
                for k in range(8):
                    e2_col = _ap_with(E2[:, k:k + 1], [[0, 128]])
                    nc.tensor.matmul(pS[:, :], e2_col, sel4b[:, :],
                                     start=(k == 0), stop=(k == 7))
                Rb = temps.tile([128, BS], F32, tag="Rb")
                nc.vector.reciprocal(Rb[:, :], pS[:, :])
                pctx = psum.tile([128, BS], F32, tag="ctx")
                for k in range(8):
                    nc.tensor.matmul(
                        pctx[:, :], encN[:, k * 128:(k + 1) * 128],
                        abuf[:, k * BS:(k + 1) * BS],
                        start=(k == 0), stop=(k == 7),
                    )
                ctx_sb = temps.tile([128, BS], BF16, tag="ctxsb")
                nc.vector.tensor_mul(ctx_sb[:, :], pctx[:, :], Rb[:, :])

                # --- phase C: gates + LSTM pointwise ---
                # col order (g, i, f, o); the 0.5 tanh-input scale for i,f,o is
                # folded into whhT/wcT/waug rows host-side.
                pg = psum.tile([128, 4 * BS], F32, tag="g")
                for m in range(4):
                    sl = pg[:, m * BS:(m + 1) * BS]
                    if first:
                        nc.tensor.matmul(sl, wcT[:, m * 128:(m + 1) * 128], ctx_sb[:, :],
                                         start=True, stop=False)
                    else:
                        nc.tensor.matmul(sl, whhT[:, m * 128:(m + 1) * 128], H[:, :],
                                         start=True, stop=False)
                        nc.tensor.matmul(sl, wcT[:, m * 128:(m + 1) * 128], ctx_sb[:, :],
                                         start=False, stop=False)
                    nc.tensor.matmul(sl, waug[:, m * 128:(m + 1) * 128],
                                     yaug[:, t * BS:(t + 1) * BS],
                                     start=False, stop=True)
                # tanh of (g, i, f) in one shot; tanh(o) separate so the DVE
                # chain below overlaps it on the ACT engine
                T3 = temps.tile([128, 3 * BS], F32, tag="T3")
                nc.scalar.activation(T3[:, :], pg[:, 0:3 * BS], AF.Tanh)
                To = temps.tile([128, BS], F32, tag="To")
                nc.scalar.activation(To[:, :], pg[:, 3 * BS:4 * BS], AF.Tanh)
                u = temps.tile([128, BS], F32, tag="u")
                nc.vector.scalar_tensor_tensor(
                    out=u[:, :], in0=T3[:, BS:2 * BS], scalar=1.0, in1=T3[:, 0:BS],
                    op0=OP.add, op1=OP.mult)  # (t_i+1)*t_g = 2*sig(i)*g~
                if first:
                    w2 = u
                else:
                    v = temps.tile([128, BS], F32, tag="v")
                    nc.vector.scalar_tensor_tensor(
                        out=v[:, :], in0=T3[:, 2 * BS:3 * BS], scalar=1.0, in1=Cn[:, :],
                        op0=OP.add, op1=OP.mult)  # (t_f+1)*c = 2*sig(f)*c
                    w2 = temps.tile([128, BS], F32, tag="w2t")
                    nc.vector.tensor_add(w2[:, :], u[:, :], v[:, :])  # 2*c_new
                # tanh(c') straight from 2c' (scale=0.5); Cn/Cb updates run
                # off the critical chain in parallel with th/H
                th = temps.tile([128, BS], F32, tag="th")
                nc.scalar.activation(th[:, :], w2[:, :], AF.Tanh, scale=0.5)
                nc.vector.tensor_scalar(out=Cn[:, :], in0=w2[:, :], scalar1=0.5,
                                        scalar2=None, op0=OP.mult)
                nc.vector.tensor_scalar(out=Cb[:, :], in0=w2[:, :], scalar1=0.5,
                                        scalar2=None, op0=OP.mult)
                nc.vector.scalar_tensor_tensor(
                    out=H[:, :], in0=To[:, :], scalar=1.0, in1=th[:, :],
                    op0=OP.add, op1=OP.mult)  # (t_o+1)*tanh(c) = 2*h_new

            # ---- final output ----
            po = psum.tile([1, BS], F32, tag="o")
            nc.tensor.matmul(po[:, :], fcf[:, 0:1], H[:, :], start=True, stop=False)
            nc.tensor.matmul(po[:, :], fcf[:, 1:2], ctx_sb[:, :], start=False, stop=True)
            fcfb = consts.tile([1, 1], F32)
            nc.sync.dma_start(out=fcfb, in_=d_fcfb[:, :])
            out_sb = temps.tile([1, BS], F32, tag="osb")
            nc.vector.scalar_tensor_tensor(
                out=out_sb[:, :], in0=po[:, :], scalar=fcfb[:, :], in1=ylast[:, :],
                op0=OP.add, op1=OP.add)
            nc.sync.dma_start(out=d_out[:, :], in_=out_sb[:, :])

    nc.compile()
    return nc


def _prep_inputs(input_encoded, y_history, attn_W1, attn_b1, attn_W2, attn_b2,
                 W_ih, W_hh, b_ih, b_hh, fc_W, fc_b, fcf_W, fcf_b):
    """Host-side weight fusion + per-core shard layout prep (numpy only)."""
    f32 = np.float32
    bf16 = ml_dtypes.bfloat16
    input_encoded = np.asarray(input_encoded, f32)
    y_history = np.asarray(y_history, f32)

    # attention weights
    W1 = np.asarray(attn_W1, f32)            # [128, 384] cols: h, c, enc
    w1hT = np.ascontiguousarray((0.5 * W1[:, 0:128]).T)     # H = 2h
    w1cT = np.ascontiguousarray(W1[:, 128:256].T)
    w1eT = np.ascontiguousarray(W1[:, 256:384].T)
    b1 = np.asarray(attn_b1, f32).reshape(128, 1)
    w2c = np.asarray(attn_W2, f32).reshape(1, 128).T.copy()  # [128,1]

    # fused gate weights; reorder (i,f,g,o) -> (g,i,f,o); fold the 0.5
    # sigmoid-as-tanh input scale into the i,f,o rows so one tanh covers
    # all four gates
    W_ih = np.asarray(W_ih, f32)
    W_hh = np.asarray(W_hh, f32)
    fc_W = np.asarray(fc_W, f32)
    wc_full = np.outer(W_ih[:, 0], fc_W[0, :128])            # [512, 128]
    w_y = W_ih[:, 0] * fc_W[0, 128]
    bias_g = np.asarray(b_ih, f32) + np.asarray(b_hh, f32) + W_ih[:, 0] * f32(fc_b[0])
    perm = np.r_[256:384, 0:128, 128:256, 384:512]
    gate_scale = np.concatenate([np.ones(128, f32), np.full(384, 0.5, f32)])
    whhT = np.ascontiguousarray((0.5 * gate_scale[:, None] * W_hh[perm]).T)  # [128, 512]
    wcT = np.ascontiguousarray((gate_scale[:, None] * wc_full[perm]).T)      # [128, 512]
    waug = np.stack([gate_scale * w_y[perm], gate_scale * bias_g[perm]], 0)  # [2, 512]

    fcf_W = np.asarray(fcf_W, f32)
    fcf = np.stack([0.5 * fcf_W[0, 0:128], fcf_W[0, 128:256]], 1)  # [128, 2]
    fcfb = np.array([[np.asarray(fcf_b, f32).reshape(-1)[0]]], f32)

    # selection matrix: sel4[r, b] = (r % 32 == b)
    r = np.arange(128)
    sel4 = (np.equal.outer(r % BS, np.arange(BS))).astype(f32)  # [128, 32]

    shared = dict(
        w1eT=w1eT, b1=b1,
        w1hT=w1hT.astype(bf16), w1cT=w1cT.astype(bf16), w2c=w2c.astype(bf16),
        sel4b=sel4.astype(bf16),
        whhT=whhT.astype(bf16), wcT=wcT.astype(bf16), waug=waug.astype(bf16),
        fcf=fcf.astype(bf16), fcfb=fcfb,
    )

    in_maps = []
    for c in range(NCORES):
        enc_c = input_encoded[c * BS:(c + 1) * BS]           # [32, 32, 128]
        y_c = y_history[c * BS:(c + 1) * BS, :, 0]           # [32b, 32tau]
        encT = np.ascontiguousarray(enc_c.transpose(2, 1, 0).reshape(ENC, T * BS))
        # encN[r, k*128+e] = enc[b=r%32, tau=4k+r//32, e]
        tmp = enc_c.transpose(1, 0, 2).reshape(8, 4, BS, ENC)   # [k, tau_lo, b, e]
        encN = np.ascontiguousarray(tmp.transpose(1, 2, 0, 3).reshape(128, 8 * ENC))
        yrow = np.ascontiguousarray(y_c.T.reshape(1, T * BS))   # [1, tau*32+b]
        yaug = np.concatenate([yrow, np.ones_like(yrow)], 0)    # [2, 1024]
        m = dict(shared)
        m.update(
            encT=encT, encN=encN.astype(bf16), yaug=yaug.astype(bf16),
            ylast=np.ascontiguousarray(y_c[:, T - 1].reshape(1, BS)),
        )
        in_maps.append(m)
    return in_maps


_CACHED = {}


def kernel(**inputs) -> np.ndarray:
    in_maps = _prep_inputs(**inputs)
    if "nc" not in _CACHED:
        _CACHED["nc"] = build_program()
    res = run_bass_kernel_spmd(_CACHED["nc"], in_maps, core_ids=list(range(NCORES)))
    out = np.concatenate([r["outp"].reshape(BS, 1) for r in res.results], 0)
    return out.astype(np.float32)


if __name__ == "__main__":
    import reference
    inputs = {k: np.asarray(v) for k, v in reference.setup_inputs().items()}
    expected = np.asarray(reference.reference(**inputs))
    actual = kernel(**inputs)
    err = np.abs(actual - expected).max() / (np.abs(expected).max() + 1e-12)
    print("Relative error:", err)
